# revision 23
# baseline (speedup 1.0000x reference)
"""NT-Xent loss on 8 Trainium2 cores (v5: dual-engine fused exp).

Structure (per core (v,s), cyclic 3-block symmetry as v4): slab s's
1024 rows against column blocks [s | s+2 | s+1] (3072 cols).  The v4
bottleneck was ScalarE doing ALL 3.1M exps (25us) while DVE spent 23us
on reductions.  v5 splits the exp work across both engines, each with a
FUSED free-dim accumulator, and moves the +1-block column sums to the
PE:

- psB half [+2b 512 | +1 1024]: ScalarE ACT Exp -> fp8, accum_out =
  rowsum (pre-quantization fp32, verified on HW).
- psA half [own 1024 | +2a 512]: custom DVE op EXP8_SUM_ANT
  (body = sq(sq(sq((x*c0 + c1)*x + c2))) ~= lam*e^(x*ASC), fitted;
  7 ALU stages + accum) -> fp8 scratch, accum_out = rowsum.
- +1-block colsums: paired DoubleRow ones-matmuls (ones [128,2,128]
  weight, rhs = two i-tiles' fp8 exp) accumulated across the 4 i-tile
  pairs into 2 persistent PSUM banks (128-replicated rows).

Host: rowsum = (accD/lam + accA) - diag + colsum[(s-1)%4]; diag is
emulated exactly (EXP8 is bit-exact vs np.float32 emulation, verified
on HW).  Shift sigma=0 so exp values are fp8-friendly (e^s, s in
[-3.3, 3.3] off-diagonal); lse = ln(rowsum + e^pos).
"""

import numpy as np
import ml_dtypes

N = 4096
D = 256
TEMP = 0.1
NCORES = 8
RPC = 2 * N // NCORES          # 1024 rows per core
IT = RPC // 128                # 8 i-tiles of 128 rows
W = 3 * RPC                    # 3072 columns per core
HALFW = W // 2                 # 1536 cols per PSUM buffer
NCH = HALFW // 512             # 3 column chunks per half
SC = 16.0                      # fp8 prescale (power of 2, exact)
ASCALE = (1.0 / TEMP) / (SC * SC)   # 10/256, exact in fp32

# EXP8 fit: (C2S + C1S*s + C0S*s^2)^8 ~= LAM * e^s  (s = G*ASCALE),
# weighted by N(0,0.625)*e^s over s in [-3.45, 3.45]
C0S, C1S, C2S = 0.00812527624, 0.125296963, 0.999881204
LAM = 1.00007132
C0G = float(np.float32(C0S * ASCALE * ASCALE))
C1G = float(np.float32(C1S * ASCALE))
C2G = float(np.float32(C2S))

# per-tile +2 split: ACT takes [0:G2T[t]], DVE the rest; chosen so each
# tile's engine load is balanced as the own-triangle width 128*(t+1) ramps
G2T = (0, 32, 96, 176, 240, 320, 384, 464)

_CACHE = {}


def _register_exp8():
    """Register the EXP8_SUM_ANT custom DVE op (runtime equivalent of the
    documented OPS.append flow; sha computed from the lowered uops)."""
    from operator import add
    from concourse.dve_spec import Spec, Src0, C0, C1, C2, Zero, sq, lower
    from concourse.dve_uop import DveOpSpec
    import concourse.dve_ops as dom

    name = "EXP8_SUM_ANT"
    for op in dom.OPS:
        if op.name == name:
            return op

    body = sq(sq(sq((Src0 * C0 + C1) * Src0 + C2)))

    def ref(in0, in1, s0, s1, imm2):
        x = in0.astype(np.float32)
        t = ((x * np.float32(s0) + np.float32(s1)) * x + np.float32(imm2)).astype(
            np.float32
        )
        t = (t * t).astype(np.float32)
        t = (t * t).astype(np.float32)
        t = (t * t).astype(np.float32)
        return t, t.reshape(t.shape[0], -1).sum(axis=-1, keepdims=True).astype(
            np.float32
        )

    spec = Spec(body=body, accum=add, accum_init=Zero, reference=ref)
    row = dom._CUSTOM_DVE_ROW_BASE + len(dom.OPS)
    dom._SUB_OPCODE_FOR_NAME[name] = row
    shas = {}
    for ver in ("v3", "v4"):
        shas[ver] = DveOpSpec(
            name=name, opcode=row, uops=lower(spec, ver=ver), rd1_en=False
        ).sha(ver)
    op = dom.DveOp(name, spec, subdim=False, uops_sha=shas)
    dom.OPS.append(op)
    dom.CUSTOM_DVE_SPECS[name] = spec
    return op


def _exp8_host(x):
    """Bit-exact host emulation of the device EXP8 body (fp32 stages)."""
    x = np.asarray(x, dtype=np.float32)
    t = ((x * np.float32(C0G) + np.float32(C1G)) * x + np.float32(C2G)).astype(
        np.float32
    )
    t = (t * t).astype(np.float32)
    t = (t * t).astype(np.float32)
    t = (t * t).astype(np.float32)
    return t


def _build_program():
    if "nc" in _CACHE:
        return _CACHE["nc"]

    import concourse.tile as tile
    from concourse import bacc, mybir

    EXP8 = _register_exp8()

    F8 = mybir.dt.float8e4
    F32 = mybir.dt.float32

    nc = bacc.Bacc(
        "TRN2", target_bir_lowering=False, debug=False, num_devices=NCORES
    )

    # anT[h][c][p][k][col] = cols[h*1536 + c*512 + col, k*128 + p]
    # column order per core: [own 1024 | +2 1024 | +1 1024]
    anT_d = nc.dram_tensor("anT", [2, NCH, 128, 2, 512], F8, kind="ExternalInput")
    acc_d = nc.dram_tensor("acc", [128, IT, 4], F32, kind="ExternalOutput")
    e1_d = nc.dram_tensor("e1", [128, IT, 1024], F8, kind="ExternalOutput")
    eo_d = nc.dram_tensor("eo", [128, IT, 1024], F8, kind="ExternalOutput")

    with tile.TileContext(nc) as tc:
        with (
            tc.tile_pool(name="weights", bufs=1) as wpool,
            tc.tile_pool(name="psum", bufs=4, space="PSUM") as ppool,
        ):
            an = [
                [wpool.tile([128, 2, 512], F8, name=f"an{h}_{c}") for c in range(NCH)]
                for h in range(2)
            ]
            # block -> an tiles: own = an[0][0..1], +2 = an[0][2], an[1][0],
            # +1 = an[1][1..2].  The own block IS the slab, so lhsT slices
            # come from an[0][0..1] directly (no separate qnT transfer).
            # First-use order per tile: own, +2, +1.
            nc.scalar.dma_start(out=an[0][0][:], in_=anT_d[0, 0])
            nc.sync.dma_start(out=an[1][1][:], in_=anT_d[1, 1])
            nc.gpsimd.dma_start(out=an[1][2][:], in_=anT_d[1, 2])
            nc.scalar.dma_start(out=an[0][1][:], in_=anT_d[0, 1])
            nc.sync.dma_start(out=an[0][2][:], in_=anT_d[0, 2])
            nc.gpsimd.dma_start(out=an[1][0][:], in_=anT_d[1, 0])

            acc = wpool.tile([128, IT, 4], F32)
            scrA = wpool.tile([128, 1024], F8)
            eofull = wpool.tile([128, IT, 1024], F8)
            e1full = wpool.tile([128, IT, 1024], F8)
            w1 = wpool.tile([128, 2, 128], F8)
            nc.vector.memset(w1[:], 1.0)

            AN = an[0] + an[1]  # flat list of 6 [128,2,512] tiles

            cs = None
            warm = None
            for t in range(IT):
                lhsT = AN[t // 4][:, :, (t % 4) * 128:(t % 4) * 128 + 128]
                # chunk order [+1, own, +2]: ACT (the longer chain) is fed
                # first, and the +1 exps finish early for the colsum MMs
                # psB: two slots for the +1 chunks (read by both engines,
                # 2-tile slack).  psA: own + +2 alternate slots per tile, so
                # each slot's previous reader is the OTHER engine's early op
                # and neither serial chain absorbs an MM-fill bubble.
                psB1 = ppool.tile([128, 1024], F32, tag="psB", bufs=2)
                psX = ppool.tile([128, 1024], F32, tag="psA", bufs=2)
                psY = ppool.tile([128, 1024], F32, tag="psA", bufs=2)
                psO, ps2 = (psX, psY) if t % 2 == 0 else (psY, psX)
                wd = (t + 1) * 128
                g = G2T[t]
                # MM order [own, +1, +2]: each engine's ops then match
                # their input-arrival order (DVE: own,+1b; ACT: +1a,+2)
                for ps, blk, w in ((psO, 0, wd), (psB1, 2, 1024), (ps2, 1, 1024)):
                    if t == 0 and blk == 0:
                        with tc.high_priority():
                            for _ in range(24):
                                nc.tensor.matmul(
                                    ps[:, 0:128],
                                    w1[:],
                                    w1[:],
                                    start=True,
                                    stop=True,
                                    perf_mode=mybir.MatmulPerfMode.DoubleRow,
                                    skip_group_check=True,
                                )
                    for k in range(2):
                        if k * 512 >= w:
                            break
                        kw = min(w - k * 512, 512)
                        nc.tensor.matmul(
                            ps[:, k * 512:k * 512 + kw],
                            lhsT,
                            AN[2 * blk + k][:, :, 0:kw],
                            start=True,
                            stop=True,
                            perf_mode=mybir.MatmulPerfMode.DoubleRow,
                            skip_group_check=True,
                        )
                # own block (triangle) -> DVE EXP8; diagonal emulated on host
                nc.vector._custom_dve(
                    EXP8,
                    out=eofull[:, t, 0:wd],
                    in0=psO[:, 0:wd],
                    s0=C0G,
                    s1=C1G,
                    imm2=C2G,
                    accum_out=acc[:, t, 1:2],
                )
                # +1 block: split between engines (kept for host colsums)
                if g > 0:
                    nc.scalar.activation(
                        e1full[:, t, 0:g],
                        psB1[:, 0:g],
                        mybir.ActivationFunctionType.Exp,
                        bias=0.0,
                        scale=float(ASCALE),
                        accum_out=acc[:, t, 0:1],
                    )
                nc.vector._custom_dve(
                    EXP8,
                    out=e1full[:, t, g:1024],
                    in0=psB1[:, g:1024],
                    s0=C0G,
                    s1=C1G,
                    imm2=C2G,
                    accum_out=acc[:, t, 3:4],
                )
                # +2 block -> ScalarE whole
                nc.scalar.activation(
                    scrA[:],
                    ps2[:],
                    mybir.ActivationFunctionType.Exp,
                    bias=0.0,
                    scale=float(ASCALE),
                    accum_out=acc[:, t, 2:3],
                )
                if t % 2 == 1:
                    # ship the pair's +1/own exp values; colsums on host
                    nc.sync.dma_start(
                        out=e1_d[:, t - 1:t + 1], in_=e1full[:, t - 1:t + 1]
                    )
                    nc.sync.dma_start(
                        out=eo_d[:, t - 1:t + 1, 0:(t + 1) * 128],
                        in_=eofull[:, t - 1:t + 1, 0:(t + 1) * 128],
                    )
                if t == IT - 2:
                    nc.sync.dma_start(
                        out=acc_d[:, 0:IT - 1], in_=acc[:, 0:IT - 1]
                    )


            nc.sync.dma_start(out=acc_d[:, IT - 1:IT], in_=acc[:, IT - 1:IT])

    nc.compile()
    _CACHE["nc"] = nc
    return nc


def _prep_inputs(z_i, z_j):
    f8 = ml_dtypes.float8_e4m3
    zin = z_i / np.sqrt(np.sum(z_i * z_i, axis=1, keepdims=True))
    zjn = z_j / np.sqrt(np.sum(z_j * z_j, axis=1, keepdims=True))
    posn = np.sum(zin * zjn, axis=1, dtype=np.float64) / TEMP      # [4096]

    q8 = [(SC * zjn).astype(f8), (SC * zin).astype(f8)]
    # exact squared norms of the quantized rows: the device Gram diagonal
    dsq = [np.sum(b.astype(np.float64) ** 2, axis=1) for b in q8]

    in_maps = []
    for c in range(NCORES):
        v, s = divmod(c, NCORES // 2)
        b = q8[v]
        brot = np.roll(b, -s * RPC, axis=0)
        # column order: [own | +2 | +1]; +1 sits in psB at local cols
        # 512:1536 so the ones-MMs read e1[:, :, 512:1536]
        cols = np.concatenate(
            [brot[0:RPC], brot[2 * RPC:3 * RPC], brot[RPC:2 * RPC]], axis=0
        )                                               # [3072, 256]
        anT = np.ascontiguousarray(
            cols.T.reshape(2, 128, 2, NCH, 512).transpose(2, 3, 1, 0, 4)
        )
        in_maps.append({"anT": anT})
    return in_maps, posn, dsq


def kernel(z_i, z_j):
    z_i = np.asarray(z_i, dtype=np.float32)
    z_j = np.asarray(z_j, dtype=np.float32)

    from concourse.bass_utils import run_bass_kernel_spmd

    nc = _build_program()
    in_maps, posn, dsq = _prep_inputs(z_i, z_j)

    res = run_bass_kernel_spmd(nc, in_maps, list(range(NCORES)))
    _CACHE["last_results"] = res

    nv = NCORES // 2
    rowsum = np.empty(2 * N, dtype=np.float64)
    colsum = np.empty((2, nv, RPC), dtype=np.float64)
    for c in range(NCORES):
        v, s = divmod(c, nv)
        a = res.results[c]["acc"].astype(np.float64)   # [128, IT, 4]
        for t in range(IT):
            if G2T[t] == 0:
                a[:, t, 0] = 0.0   # no ACT +1 op on this tile
        # slots: 0=+1a (ACT), 1=own (EXP8), 2=+2 (ACT), 3=+1b (EXP8)
        rs = a[:, :, 0] + a[:, :, 2] + (a[:, :, 1] + a[:, :, 3]) / LAM
        rowsum[c * RPC:(c + 1) * RPC] = rs.T.reshape(-1)
        e1 = res.results[c]["e1"]
        if e1.dtype != np.dtype(ml_dtypes.float8_e4m3):
            e1 = e1.view(ml_dtypes.float8_e4m3)
        colsum[v, s] = e1.astype(np.float32).astype(np.float64).sum(axis=(0, 1))
        # own-block upper-triangle: row r of i-tile t gets the colsums of
        # its column in every later tile's computed prefix
        eo = res.results[c]["eo"]
        if eo.dtype != np.dtype(ml_dtypes.float8_e4m3):
            eo = eo.view(ml_dtypes.float8_e4m3)
        ecs = eo.astype(np.float32).astype(np.float64).sum(axis=0)  # [IT, 1024]
        upper = np.zeros(RPC)
        run = np.zeros(RPC)
        for t in range(IT - 1, -1, -1):
            upper[t * 128:(t + 1) * 128] = run[t * 128:(t + 1) * 128]
            run += ecs[t]  # tile t computed cols [0:128*(t+1)]
        rowsum[c * RPC:(c + 1) * RPC] += upper
    for v in range(2):
        for s in range(nv):
            g0 = v * N + s * RPC
            rowsum[g0:g0 + RPC] += colsum[v, (s - 1) % nv]

    # exact diagonal removal: the diagonal sits in the own block (EXP8);
    # emulate the device computation bit-exactly
    dsq_g = np.concatenate(dsq).astype(np.float32)     # [8192] |q8 row|^2
    rowsum -= _exp8_host(dsq_g).astype(np.float64) / LAM

    posn_g = np.concatenate([posn, posn])
    epos_g = np.exp(posn_g)

    lse = np.log(rowsum + epos_g)
    loss = np.mean(lse - posn_g)
    return np.array(loss, dtype=np.float32)


# revision 24
# speedup vs baseline: 1.1710x; 1.1710x over previous
"""NT-Xent loss on 8 Trainium2 cores (v5: dual-engine fused exp).

Structure (per core (v,s), cyclic 3-block symmetry as v4): slab s's
1024 rows against column blocks [s | s+2 | s+1] (3072 cols).  The v4
bottleneck was ScalarE doing ALL 3.1M exps (25us) while DVE spent 23us
on reductions.  v5 splits the exp work across both engines, each with a
FUSED free-dim accumulator, and moves the +1-block column sums to the
PE:

- psB half [+2b 512 | +1 1024]: ScalarE ACT Exp -> fp8, accum_out =
  rowsum (pre-quantization fp32, verified on HW).
- psA half [own 1024 | +2a 512]: custom DVE op EXP8_SUM_ANT
  (body = sq(sq(sq((x*c0 + c1)*x + c2))) ~= lam*e^(x*ASC), fitted;
  7 ALU stages + accum) -> fp8 scratch, accum_out = rowsum.
- +1-block colsums: paired DoubleRow ones-matmuls (ones [128,2,128]
  weight, rhs = two i-tiles' fp8 exp) accumulated across the 4 i-tile
  pairs into 2 persistent PSUM banks (128-replicated rows).

Host: rowsum = (accD/lam + accA) - diag + colsum[(s-1)%4]; diag is
emulated exactly (EXP8 is bit-exact vs np.float32 emulation, verified
on HW).  Shift sigma=0 so exp values are fp8-friendly (e^s, s in
[-3.3, 3.3] off-diagonal); lse = ln(rowsum + e^pos).
"""

import numpy as np
import ml_dtypes

N = 4096
D = 256
TEMP = 0.1
NCORES = 8
RPC = 2 * N // NCORES          # 1024 rows per core
IT = RPC // 128                # 8 i-tiles of 128 rows
W = 3 * RPC                    # 3072 columns per core
HALFW = W // 2                 # 1536 cols per PSUM buffer
NCH = HALFW // 512             # 3 column chunks per half
SC = 16.0                      # fp8 prescale (power of 2, exact)
ASCALE = (1.0 / TEMP) / (SC * SC)   # 10/256, exact in fp32

# EXP8 fit: (C2S + C1S*s + C0S*s^2)^8 ~= LAM * e^s  (s = G*ASCALE),
# weighted by N(0,0.625)*e^s over s in [-3.45, 3.45]
C0S, C1S, C2S = 0.00812527624, 0.125296963, 0.999881204
LAM = 1.00007132
C0G = float(np.float32(C0S * ASCALE * ASCALE))
C1G = float(np.float32(C1S * ASCALE))
C2G = float(np.float32(C2S))

# per-tile +2 split: ACT takes [0:G2T[t]], DVE the rest; chosen so each
# tile's engine load is balanced as the own-triangle width 128*(t+1) ramps
G2T = (0, 32, 96, 176, 240, 320, 384, 464)

_CACHE = {}


def _register_exp8():
    """Register the EXP8_SUM_ANT custom DVE op (runtime equivalent of the
    documented OPS.append flow; sha computed from the lowered uops)."""
    from operator import add
    from concourse.dve_spec import Spec, Src0, C0, C1, C2, Zero, sq, lower
    from concourse.dve_uop import DveOpSpec
    import concourse.dve_ops as dom

    name = "EXP8_SUM_ANT"
    for op in dom.OPS:
        if op.name == name:
            return op

    body = sq(sq(sq((Src0 * C0 + C1) * Src0 + C2)))

    def ref(in0, in1, s0, s1, imm2):
        x = in0.astype(np.float32)
        t = ((x * np.float32(s0) + np.float32(s1)) * x + np.float32(imm2)).astype(
            np.float32
        )
        t = (t * t).astype(np.float32)
        t = (t * t).astype(np.float32)
        t = (t * t).astype(np.float32)
        return t, t.reshape(t.shape[0], -1).sum(axis=-1, keepdims=True).astype(
            np.float32
        )

    spec = Spec(body=body, accum=add, accum_init=Zero, reference=ref)
    row = dom._CUSTOM_DVE_ROW_BASE + len(dom.OPS)
    dom._SUB_OPCODE_FOR_NAME[name] = row
    shas = {}
    for ver in ("v3", "v4"):
        shas[ver] = DveOpSpec(
            name=name, opcode=row, uops=lower(spec, ver=ver), rd1_en=False
        ).sha(ver)
    op = dom.DveOp(name, spec, subdim=False, uops_sha=shas)
    dom.OPS.append(op)
    dom.CUSTOM_DVE_SPECS[name] = spec
    return op


def _exp8_host(x):
    """Bit-exact host emulation of the device EXP8 body (fp32 stages)."""
    x = np.asarray(x, dtype=np.float32)
    t = ((x * np.float32(C0G) + np.float32(C1G)) * x + np.float32(C2G)).astype(
        np.float32
    )
    t = (t * t).astype(np.float32)
    t = (t * t).astype(np.float32)
    t = (t * t).astype(np.float32)
    return t


def _build_program():
    if "nc" in _CACHE:
        return _CACHE["nc"]

    import concourse.tile as tile
    from concourse import bacc, mybir

    EXP8 = _register_exp8()

    F8 = mybir.dt.float8e4
    F32 = mybir.dt.float32

    nc = bacc.Bacc(
        "TRN2", target_bir_lowering=False, debug=False, num_devices=NCORES
    )

    # anT[h][c][p][k][col] = cols[h*1536 + c*512 + col, k*128 + p]
    # column order per core: [own 1024 | +2 1024 | +1 1024]
    anT_d = nc.dram_tensor("anT", [2, NCH, 128, 2, 512], F8, kind="ExternalInput")
    acc_d = nc.dram_tensor("acc", [128, IT, 4], F32, kind="ExternalOutput")
    e1_d = nc.dram_tensor("e1", [128, IT, 1024], F8, kind="ExternalOutput")
    eo_d = nc.dram_tensor("eo", [128, IT, 1024], F8, kind="ExternalOutput")

    with tile.TileContext(nc) as tc:
        with (
            tc.tile_pool(name="weights", bufs=1) as wpool,
            tc.tile_pool(name="psum", bufs=4, space="PSUM") as ppool,
        ):
            an = [
                [wpool.tile([128, 2, 512], F8, name=f"an{h}_{c}") for c in range(NCH)]
                for h in range(2)
            ]
            # block -> an tiles: own = an[0][0..1], +2 = an[0][2], an[1][0],
            # +1 = an[1][1..2].  The own block IS the slab, so lhsT slices
            # come from an[0][0..1] directly (no separate qnT transfer).
            # First-use order per tile: own, +2, +1.
            nc.scalar.dma_start(out=an[0][0][:], in_=anT_d[0, 0])
            nc.sync.dma_start(out=an[1][1][:], in_=anT_d[1, 1])
            nc.gpsimd.dma_start(out=an[1][2][:], in_=anT_d[1, 2])
            nc.scalar.dma_start(out=an[0][1][:], in_=anT_d[0, 1])
            nc.sync.dma_start(out=an[0][2][:], in_=anT_d[0, 2])
            nc.gpsimd.dma_start(out=an[1][0][:], in_=anT_d[1, 0])

            acc = wpool.tile([128, IT, 4], F32)
            scrA = wpool.tile([128, 1024], F8)
            eofull = wpool.tile([128, IT, 1024], F8)
            e1full = wpool.tile([128, IT, 1024], F8)
            w1 = wpool.tile([128, 2, 128], F8)
            nc.vector.memset(w1[:], 1.0)

            AN = an[0] + an[1]  # flat list of 6 [128,2,512] tiles

            cs = None
            warm = None
            for t in range(IT):
                lhsT = AN[t // 4][:, :, (t % 4) * 128:(t % 4) * 128 + 128]
                # chunk order [+1, own, +2]: ACT (the longer chain) is fed
                # first, and the +1 exps finish early for the colsum MMs
                # psB: two slots for the +1 chunks (read by both engines,
                # 2-tile slack).  psA: own + +2 alternate slots per tile, so
                # each slot's previous reader is the OTHER engine's early op
                # and neither serial chain absorbs an MM-fill bubble.
                psB1 = ppool.tile([128, 1024], F32, tag="psB", bufs=2)
                psX = ppool.tile([128, 1024], F32, tag="psA", bufs=2)
                psY = ppool.tile([128, 1024], F32, tag="psA", bufs=2)
                psO, ps2 = (psX, psY) if t % 2 == 0 else (psY, psX)
                wd = (t + 1) * 128
                g = G2T[t]
                # MM order [own, +2, +1]: DVE's first input lands first,
                # then ACT's, then the shared +1
                for ps, blk, w in ((psO, 0, wd), (ps2, 1, 1024), (psB1, 2, 1024)):
                    if t == 0 and blk == 0:
                        with tc.high_priority():
                            for _ in range(24):
                                nc.tensor.matmul(
                                    ps[:, 0:128],
                                    w1[:],
                                    w1[:],
                                    start=True,
                                    stop=True,
                                    perf_mode=mybir.MatmulPerfMode.DoubleRow,
                                    skip_group_check=True,
                                )
                    for k in range(2):
                        if k * 512 >= w:
                            break
                        kw = min(w - k * 512, 512)
                        nc.tensor.matmul(
                            ps[:, k * 512:k * 512 + kw],
                            lhsT,
                            AN[2 * blk + k][:, :, 0:kw],
                            start=True,
                            stop=True,
                            perf_mode=mybir.MatmulPerfMode.DoubleRow,
                            skip_group_check=True,
                        )
                # own block (triangle) -> DVE EXP8; diagonal emulated on host
                nc.vector._custom_dve(
                    EXP8,
                    out=eofull[:, t, 0:wd],
                    in0=psO[:, 0:wd],
                    s0=C0G,
                    s1=C1G,
                    imm2=C2G,
                    accum_out=acc[:, t, 1:2],
                )
                # +2 block -> ScalarE whole
                nc.scalar.activation(
                    scrA[:],
                    ps2[:],
                    mybir.ActivationFunctionType.Exp,
                    bias=0.0,
                    scale=float(ASCALE),
                    accum_out=acc[:, t, 2:3],
                )
                # +1 block: split between engines (kept for host colsums)
                if g > 0:
                    nc.scalar.activation(
                        e1full[:, t, 0:g],
                        psB1[:, 0:g],
                        mybir.ActivationFunctionType.Exp,
                        bias=0.0,
                        scale=float(ASCALE),
                        accum_out=acc[:, t, 0:1],
                    )
                nc.vector._custom_dve(
                    EXP8,
                    out=e1full[:, t, g:1024],
                    in0=psB1[:, g:1024],
                    s0=C0G,
                    s1=C1G,
                    imm2=C2G,
                    accum_out=acc[:, t, 3:4],
                )
                if t % 2 == 1:
                    # ship the pair's +1/own exp values; colsums on host
                    nc.sync.dma_start(
                        out=e1_d[:, t - 1:t + 1], in_=e1full[:, t - 1:t + 1]
                    )
                    nc.sync.dma_start(
                        out=eo_d[:, t - 1:t + 1, 0:(t + 1) * 128],
                        in_=eofull[:, t - 1:t + 1, 0:(t + 1) * 128],
                    )
                if t == IT - 2:
                    nc.sync.dma_start(
                        out=acc_d[:, 0:IT - 1], in_=acc[:, 0:IT - 1]
                    )


            nc.sync.dma_start(out=acc_d[:, IT - 1:IT], in_=acc[:, IT - 1:IT])

    nc.compile()
    _CACHE["nc"] = nc
    return nc


def _prep_inputs(z_i, z_j):
    f8 = ml_dtypes.float8_e4m3
    zin = z_i / np.sqrt(np.sum(z_i * z_i, axis=1, keepdims=True))
    zjn = z_j / np.sqrt(np.sum(z_j * z_j, axis=1, keepdims=True))
    posn = np.sum(zin * zjn, axis=1, dtype=np.float64) / TEMP      # [4096]

    q8 = [(SC * zjn).astype(f8), (SC * zin).astype(f8)]
    # exact squared norms of the quantized rows: the device Gram diagonal
    dsq = [np.sum(b.astype(np.float64) ** 2, axis=1) for b in q8]

    in_maps = []
    for c in range(NCORES):
        v, s = divmod(c, NCORES // 2)
        b = q8[v]
        brot = np.roll(b, -s * RPC, axis=0)
        # column order: [own | +2 | +1]; +1 sits in psB at local cols
        # 512:1536 so the ones-MMs read e1[:, :, 512:1536]
        cols = np.concatenate(
            [brot[0:RPC], brot[2 * RPC:3 * RPC], brot[RPC:2 * RPC]], axis=0
        )                                               # [3072, 256]
        anT = np.ascontiguousarray(
            cols.T.reshape(2, 128, 2, NCH, 512).transpose(2, 3, 1, 0, 4)
        )
        in_maps.append({"anT": anT})
    return in_maps, posn, dsq


def kernel(z_i, z_j):
    z_i = np.asarray(z_i, dtype=np.float32)
    z_j = np.asarray(z_j, dtype=np.float32)

    from concourse.bass_utils import run_bass_kernel_spmd

    nc = _build_program()
    in_maps, posn, dsq = _prep_inputs(z_i, z_j)

    res = run_bass_kernel_spmd(nc, in_maps, list(range(NCORES)))
    _CACHE["last_results"] = res

    nv = NCORES // 2
    rowsum = np.empty(2 * N, dtype=np.float64)
    colsum = np.empty((2, nv, RPC), dtype=np.float64)
    for c in range(NCORES):
        v, s = divmod(c, nv)
        a = res.results[c]["acc"].astype(np.float64)   # [128, IT, 4]
        for t in range(IT):
            if G2T[t] == 0:
                a[:, t, 0] = 0.0   # no ACT +1 op on this tile
        # slots: 0=+1a (ACT), 1=own (EXP8), 2=+2 (ACT), 3=+1b (EXP8)
        rs = a[:, :, 0] + a[:, :, 2] + (a[:, :, 1] + a[:, :, 3]) / LAM
        rowsum[c * RPC:(c + 1) * RPC] = rs.T.reshape(-1)
        e1 = res.results[c]["e1"]
        if e1.dtype != np.dtype(ml_dtypes.float8_e4m3):
            e1 = e1.view(ml_dtypes.float8_e4m3)
        colsum[v, s] = e1.astype(np.float32).astype(np.float64).sum(axis=(0, 1))
        # own-block upper-triangle: row r of i-tile t gets the colsums of
        # its column in every later tile's computed prefix
        eo = res.results[c]["eo"]
        if eo.dtype != np.dtype(ml_dtypes.float8_e4m3):
            eo = eo.view(ml_dtypes.float8_e4m3)
        ecs = eo.astype(np.float32).astype(np.float64).sum(axis=0)  # [IT, 1024]
        upper = np.zeros(RPC)
        run = np.zeros(RPC)
        for t in range(IT - 1, -1, -1):
            upper[t * 128:(t + 1) * 128] = run[t * 128:(t + 1) * 128]
            run += ecs[t]  # tile t computed cols [0:128*(t+1)]
        rowsum[c * RPC:(c + 1) * RPC] += upper
    for v in range(2):
        for s in range(nv):
            g0 = v * N + s * RPC
            rowsum[g0:g0 + RPC] += colsum[v, (s - 1) % nv]

    # exact diagonal removal: the diagonal sits in the own block (EXP8);
    # emulate the device computation bit-exactly
    dsq_g = np.concatenate(dsq).astype(np.float32)     # [8192] |q8 row|^2
    rowsum -= _exp8_host(dsq_g).astype(np.float64) / LAM

    posn_g = np.concatenate([posn, posn])
    epos_g = np.exp(posn_g)

    lse = np.log(rowsum + epos_g)
    loss = np.mean(lse - posn_g)
    return np.array(loss, dtype=np.float32)
